# revision 1
# baseline (speedup 1.0000x reference)
"""Multi-head attention (B=4, L=2048, D=1024, H=16, hd=64) on 8 Trainium2 cores.

Sharding: 8-way tensor parallel over heads. Core c owns heads (2c, 2c+1) for
all batches: it projects qkv for its heads (x replicated, w_qkv column-sliced),
runs attention, and computes a partial out-projection with its w_out row-slice.
The host sums the 8 partials (row-parallel unshard).

Per-core kernel (all matmuls in fp32r = e8m11, inputs pre-rounded on host):
  A) qkvT = W_slice.T @ x computed transposed: lhsT = W tiles, rhs = xT tiles
     -> qT/kT [128 rows = 2 heads x 64 dim, tokens] kept in SBUF (per batch);
     vT is PE-transposed into natural V layout with a ones column appended
     (the ones column makes the PV matmul emit the softmax denominator).
  B) Flash-style attention in S^T layout (keys on partitions):
     S^T = kT.T @ qT (K=64), exp on ScalarE straight from PSUM with the
     1/8 scale folded in (no max subtraction: scores are bounded ~N(0,1)),
     P^T used directly as matmul lhsT for PV -> o^T[65, i] with row 64 the
     denominator; 1/denom computed on ScalarE as exp(-ln(d)) (both functions
     share one ACT table set) and applied via a ones-matmul partition
     broadcast + DVE multiply, deferred off the PE critical path.
  C) y_partial = oT.T @ w_out_slice, streamed out per 128-token tile.

Scheduling: engine queues are in-order, so stage B's PE stream (which gates on
ScalarE exp) is padded with independent "filler" work - stage A of the next
batch, v transposes, and out-projection tiles - emitted between j iterations.
This keeps TensorE duty near 100% so the HAM clock gate stays at 2.4 GHz.
"""
import os
from collections import deque
import numpy as np
from contextlib import ExitStack

B, L, D = 4, 2048, 1024
NH, HD = 16, 64
T = B * L  # 8192 tokens
NCORES = 8
TM = 512  # stage-A token macro-tile
IM = 1024  # stage-B query macro-tile


def _round_fp32r(a: np.ndarray) -> np.ndarray:
    """Round fp32 to fp32r (e8m11: fp32 with low 12 mantissa bits zero), RNE."""
    u = np.ascontiguousarray(a, dtype=np.float32).view(np.uint32).copy()
    add = np.uint32(0x7FF) + ((u >> np.uint32(12)) & np.uint32(1))
    u = (u + add) & np.uint32(0xFFFFF000)
    return u.view(np.float32)


def _build_program():
    import concourse.bacc as bacc
    import concourse.tile as tile
    from concourse import mybir

    F32 = mybir.dt.float32
    F32R = mybir.dt.float32r
    EXP = mybir.ActivationFunctionType.Exp

    nc = bacc.Bacc(
        "TRN2", target_bir_lowering=False, debug=False, num_devices=NCORES
    )
    xT_d = nc.dram_tensor("xT", [D, T], F32R, kind="ExternalInput")
    wqkv_d = nc.dram_tensor("wqkv", [D, 384], F32R, kind="ExternalInput")
    wout_d = nc.dram_tensor("wout", [128, D], F32R, kind="ExternalInput")
    ones_d = nc.dram_tensor("ones", [128, 64], F32R, kind="ExternalInput")
    ident_d = nc.dram_tensor("ident", [128, 128], F32, kind="ExternalInput")
    y_d = nc.dram_tensor("y", [T, D], F32, kind="ExternalOutput")

    xT_v = xT_d.ap().rearrange("(k p) t -> p k t", p=128)  # [128, 8, T]
    wqkv_v = wqkv_d.ap().rearrange("(k p) c -> p k c", p=128)  # [128, 8, 384]

    NTM = L // TM  # stage-A macro tiles per batch
    NJ = L // 128  # key tiles per batch
    NIM = L // IM  # query macro tiles per batch

    with tile.TileContext(nc) as tc, ExitStack() as ctx:
        consts = ctx.enter_context(tc.tile_pool(name="consts", bufs=1))
        sb_x = ctx.enter_context(tc.tile_pool(name="sb_x", bufs=2))
        sb_qk = ctx.enter_context(tc.tile_pool(name="sb_qk", bufs=2))
        sb_v = ctx.enter_context(tc.tile_pool(name="sb_v", bufs=2))
        sb_vst = ctx.enter_context(tc.tile_pool(name="sb_vst", bufs=5))
        sb_p = ctx.enter_context(tc.tile_pool(name="sb_p", bufs=3))
        sb_o = ctx.enter_context(tc.tile_pool(name="sb_o", bufs=2))
        sb_oT = ctx.enter_context(tc.tile_pool(name="sb_oT", bufs=2))
        sb_y = ctx.enter_context(tc.tile_pool(name="sb_y", bufs=3))
        VAR = os.environ.get("ATTN_KERNEL_VARIANT", "d")
        if VAR in ("a", "b", "d"):
            ps_s = ctx.enter_context(
                tc.tile_pool(name="ps_s", bufs=2, space="PSUM")
            )
            ps_po = ctx.enter_context(
                tc.tile_pool(name="ps_po", bufs=1, space="PSUM")
            )
            ps_m = ctx.enter_context(
                tc.tile_pool(name="ps_m", bufs=2, space="PSUM")
            )
        else:
            ps_m = ctx.enter_context(
                tc.tile_pool(name="ps_m", bufs=2, space="PSUM")
            )
            ps_po = ctx.enter_context(
                tc.tile_pool(name="ps_po", bufs=2, space="PSUM")
            )
            ps_s = ps_m

        wq_t = consts.tile([128, 8, 384], F32R, tag="wqkv")
        nc.sync.dma_start(wq_t[:], wqkv_v[:])
        wo_t = consts.tile([128, D], F32R, tag="wout")
        nc.sync.dma_start(wo_t[:], wout_d[:])
        ones_t = consts.tile([128, 64], F32R, tag="ones")
        nc.sync.dma_start(ones_t[:], ones_d[:])
        ident_t = consts.tile([128, 128], F32, tag="ident")
        nc.sync.dma_start(ident_t[:], ident_d[:])

        qk_tiles = {}  # b -> (qT, kT, v_aug)

        def stage_a_units(b):
            """Return emitter closures for batch b's qkv projection."""
            qT_b = sb_qk.tile([128, L], F32R, tag="qT")
            kT_b = sb_qk.tile([128, L], F32R, tag="kT")
            v_b = sb_v.tile([128, 2, NJ, 65], F32R, tag="v")
            qk_tiles[b] = (qT_b, kT_b, v_b)
            xt_tiles = {}
            vst_tiles = {}

            def ones_col():
                nc.vector.tensor_copy(
                    v_b[:, :, :, 64:65],
                    ones_t[:, 0 : 2 * NJ].rearrange(
                        "p (h j o) -> p h j o", h=2, o=1
                    ),
                )

            def col_group(tm, c):
                if c == 0:
                    xt = sb_x.tile([128, 8, TM], F32R, tag="xt")
                    t0 = b * L + tm * TM
                    nc.sync.dma_start(xt[:], xT_v[:, :, t0 : t0 + TM])
                    xt_tiles[tm] = xt
                xt = xt_tiles[tm]
                psA = ps_m.tile([128, TM], mybir.dt.float32, tag="m")
                for k in range(8):
                    nc.tensor.matmul(
                        psA[:],
                        wq_t[:, k, c * 128 : (c + 1) * 128],
                        xt[:, k, :],
                        start=(k == 0),
                        stop=(k == 7),
                    )
                if c == 0:
                    nc.vector.tensor_copy(qT_b[:, tm * TM : (tm + 1) * TM], psA[:])
                elif c == 1:
                    nc.vector.tensor_copy(kT_b[:, tm * TM : (tm + 1) * TM], psA[:])
                else:
                    vst = sb_vst.tile([128, TM], mybir.dt.float32, tag="vst")
                    nc.vector.tensor_copy(vst[:], psA[:])
                    vst_tiles[tm] = vst

            def transposes(tm):
                vst = vst_tiles[tm]
                for tb in range(TM // 128):
                    jt = tm * (TM // 128) + tb
                    ptr = ps_m.tile([128, 128], mybir.dt.float32, tag="m")
                    nc.tensor.transpose(
                        ptr[:], vst[:, tb * 128 : (tb + 1) * 128], ident_t[:]
                    )
                    nc.vector.tensor_copy(v_b[:, 0, jt, 0:64], ptr[:, 0:64])
                    nc.vector.tensor_copy(v_b[:, 1, jt, 0:64], ptr[:, 64:128])

            units = [(0.1, ones_col)]
            for tm in range(NTM):
                for c in range(3):
                    units.append((1.9, lambda tm=tm, c=c: col_group(tm, c)))
                units.append((0.8, lambda tm=tm: transposes(tm)))
            return units

        filler = deque()  # batch-deadline units (stage A, norm tails)

        pending_norm = []
        pace = {"credit": 0.0, "scredit": 0.0, "g_iters": 1}

        def pop_filler():
            """Cost-weighted pacing: spread queued filler evenly over the
            batch's j iterations instead of draining it front-loaded.
            Returns the PE-cost (us) emitted this call."""
            total = sum(c for c, _ in filler)
            iters_left = pace.get("iters_left", 1)
            rate = total / max(iters_left, 1)
            pace["credit"] += rate
            done = 0.0
            while filler and pace["credit"] >= filler[0][0] * 0.5:
                c, fn = filler.popleft()
                pace["credit"] -= c
                done += c
                fn()
            pace["iters_left"] = max(iters_left - 1, 1)
            return done

        def stage_b_single(b):
            """v3a structure: one head at a time, ps_o bufs=1."""
            qT_b, kT_b, v_b = qk_tiles[b]
            pace["iters_left"] = NIM * 2 * NJ
            for im in range(NIM):
                oT_b = sb_oT.tile([128, IM], F32R, tag="oT")
                for h in range(2):
                    hb = h * 64
                    po = ps_po.tile([65, IM], mybir.dt.float32, tag="po")
                    p_prev = None
                    for j in range(NJ):
                        ps = ps_s.tile([128, IM], mybir.dt.float32, tag="s")
                        lhsT = kT_b[hb : hb + 64, j * 128 : (j + 1) * 128]
                        for q2 in range(IM // 512):
                            i0 = im * IM + q2 * 512
                            nc.tensor.matmul(
                                ps[:, q2 * 512 : (q2 + 1) * 512],
                                lhsT,
                                qT_b[hb : hb + 64, i0 : i0 + 512],
                                start=True,
                                stop=True,
                            )
                        if j == 0 and pending_norm:
                            for fn in pending_norm:
                                fn()
                            pending_norm.clear()
                        p_t = sb_p.tile([128, IM], F32R, tag="p")
                        nc.scalar.activation(p_t[:], ps[:], EXP, scale=0.125)
                        if p_prev is not None:
                            jp, pp = p_prev
                            for q2 in range(IM // 512):
                                nc.tensor.matmul(
                                    po[:, q2 * 512 : (q2 + 1) * 512],
                                    v_b[:, h, jp, :],
                                    pp[:, q2 * 512 : (q2 + 1) * 512],
                                    start=(jp == 0),
                                    stop=False,
                                )
                        p_prev = (j, p_t)
                        pop_filler()
                    jp, pp = p_prev
                    for q2 in range(IM // 512):
                        nc.tensor.matmul(
                            po[:, q2 * 512 : (q2 + 1) * 512],
                            v_b[:, h, jp, :],
                            pp[:, q2 * 512 : (q2 + 1) * 512],
                            start=False,
                            stop=True,
                        )

                    def norm_d(po=po, oT_b=oT_b, hb=hb, h=h, im=im, b=b):
                        # fast part at flush: ACT ln -> exp(-x) = 1/denom
                        # (both funcs live in one ACT table set), plus the
                        # o_ev evacuation copy. The PE-side broadcast matmuls
                        # go into a deferred filler unit so the PE stream
                        # never waits on this chain.
                        o_ev = sb_o.tile([65, IM], mybir.dt.float32, tag="oe")
                        recr = sb_o.tile([65, IM], F32R, tag="recr")
                        lnrow = sb_o.tile([65, IM], mybir.dt.float32, tag="lnrow")
                        nc.scalar.activation(
                            lnrow[64:65, :], po[64:65, :],
                            mybir.ActivationFunctionType.Ln,
                        )
                        nc.scalar.activation(
                            recr[64:65, :], lnrow[64:65, :], EXP, scale=-1.0
                        )
                        nc.vector.tensor_copy(o_ev[0:64, :], po[0:64, :])

                        def tail():
                            with nc.allow_low_precision(reason="fp32r"):
                                for q2 in range(IM // 512):
                                    sl = slice(q2 * 512, (q2 + 1) * 512)
                                    pbc = ps_m.tile(
                                        [64, 512], mybir.dt.float32, tag="m"
                                    )
                                    nc.tensor.matmul(
                                        pbc[:],
                                        ones_t[64:65, :],
                                        recr[64:65, sl],
                                        start=True,
                                        stop=True,
                                    )
                                    nc.vector.tensor_mul(
                                        oT_b[hb : hb + 64, sl],
                                        o_ev[0:64, sl],
                                        pbc[:],
                                    )

                        filler.append((0.7, tail))
                        if h == 1:
                            for qi in range(IM // 256):
                                filler.append(
                                    (1.1, lambda qi=qi: proj_quarter(qi, b, im, oT_b))
                                )

                    def norm(po=po, oT_b=oT_b, hb=hb):
                        o_ev = sb_o.tile([65, IM], mybir.dt.float32, tag="oe")
                        recr = sb_o.tile([65, IM], F32R, tag="recr")
                        with nc.allow_low_precision(reason="fp32r operands"):
                            if VAR == "a":
                                # recip reads po directly, then evac copy
                                for q2 in range(IM // 512):
                                    sl = slice(q2 * 512, (q2 + 1) * 512)
                                    nc.vector.reciprocal(
                                        recr[64:65, sl], po[64:65, sl]
                                    )
                                nc.vector.tensor_copy(o_ev[0:64, :], po[0:64, :])
                            else:
                                # fast copy releases po first
                                nc.vector.tensor_copy(o_ev[:], po[:])
                                for q2 in range(IM // 512):
                                    sl = slice(q2 * 512, (q2 + 1) * 512)
                                    nc.vector.reciprocal(
                                        recr[64:65, sl], o_ev[64:65, sl]
                                    )
                            for q2 in range(IM // 512):
                                sl = slice(q2 * 512, (q2 + 1) * 512)
                                pbc = ps_m.tile(
                                    [64, 512], mybir.dt.float32, tag="m"
                                )
                                nc.tensor.matmul(
                                    pbc[:],
                                    ones_t[64:65, :],
                                    recr[64:65, sl],
                                    start=True,
                                    stop=True,
                                )
                                src = po if VAR == "a" else o_ev
                                nc.vector.tensor_mul(
                                    oT_b[hb : hb + 64, sl],
                                    o_ev[0:64, sl],
                                    pbc[:],
                                )

                    pending_norm.append(norm_d if VAR == "d" else norm)

                def proj_quarter(qi, b=b, im=im, oT_b=oT_b):
                    for ts in range(qi * 2, qi * 2 + 2):
                        y_t = sb_y.tile([128, D], mybir.dt.float32, tag="y")
                        for nh in range(2):
                            psC = ps_m.tile([128, 512], mybir.dt.float32, tag="m")
                            nc.tensor.matmul(
                                psC[:],
                                oT_b[:, ts * 128 : (ts + 1) * 128],
                                wo_t[:, nh * 512 : (nh + 1) * 512],
                                start=True,
                                stop=True,
                            )
                            nc.vector.tensor_copy(
                                y_t[:, nh * 512 : (nh + 1) * 512], psC[:]
                            )
                        t0 = b * L + im * IM + ts * 128
                        nc.sync.dma_start(y_d[t0 : t0 + 128, :], y_t[:])

                if VAR != "d":
                    for qi in range(IM // 256):
                        filler.append((1.1, lambda qi=qi: proj_quarter(qi)))

        def stage_b(b):
            if VAR in ("a", "b", "d"):
                return stage_b_single(b)
            qT_b, kT_b, v_b = qk_tiles[b]
            pace["iters_left"] = NIM * NJ
            for im in range(NIM):
                oT_b = sb_oT.tile([128, IM], F32R, tag="oT")
                po = [
                    ps_po.tile([65, IM], mybir.dt.float32, tag="po", name=f"po{_h}")
                    for _h in range(2)
                ]
                p_prev = [None, None]
                for j in range(NJ):
                    ps_t = [
                        ps_m.tile([128, IM], mybir.dt.float32, tag="m", name=f"s{_h}")
                        for _h in range(2)
                    ]
                    # q2 outer / h inner: S matmuls alternate base partition
                    # 0 / 64 so adjacent pairs co-execute in distinct PE
                    # row-groups (measured 151 ns/mm vs 236 serial)
                    for q2 in range(IM // 512):
                        i0 = im * IM + q2 * 512
                        for h in range(2):
                            hb = h * 64
                            nc.tensor.matmul(
                                ps_t[h][:, q2 * 512 : (q2 + 1) * 512],
                                kT_b[hb : hb + 64, j * 128 : (j + 1) * 128],
                                qT_b[hb : hb + 64, i0 : i0 + 512],
                                start=True,
                                stop=True,
                            )
                    if j == 0 and pending_norm:
                        # must precede this im's first PV (ps_po slot reuse)
                        for fn in pending_norm:
                            fn()
                        pending_norm.clear()
                    p_t = [None, None]
                    for h in range(2):
                        p_t[h] = sb_p.tile([128, IM], F32R, tag="p", name=f"p{h}")
                        nc.scalar.activation(
                            p_t[h][:], ps_t[h][:], EXP, scale=0.125
                        )
                    for h in range(2):
                        if p_prev[h] is not None:
                            jp, pp = p_prev[h]
                            for q2 in range(IM // 512):
                                nc.tensor.matmul(
                                    po[h][:, q2 * 512 : (q2 + 1) * 512],
                                    v_b[:, h, jp, :],
                                    pp[:, q2 * 512 : (q2 + 1) * 512],
                                    start=(jp == 0),
                                    stop=False,
                                )
                        p_prev[h] = (j, p_t[h])
                    pop_filler()
                for h in range(2):
                    jp, pp = p_prev[h]
                    for q2 in range(IM // 512):
                        nc.tensor.matmul(
                            po[h][:, q2 * 512 : (q2 + 1) * 512],
                            v_b[:, h, jp, :],
                            pp[:, q2 * 512 : (q2 + 1) * 512],
                            start=False,
                            stop=True,
                        )

                def norm(po=po, oT_b=oT_b):
                    for h in range(2):
                        hb = h * 64
                        # fast copy releases the po slot; the slow reciprocal
                        # reads the copy off the critical path
                        o_ev = sb_o.tile([65, IM], mybir.dt.float32, tag="oe")
                        nc.vector.tensor_copy(o_ev[:], po[h][:])
                        recr = sb_o.tile([65, IM], F32R, tag="recr")
                        with nc.allow_low_precision(reason="fp32r operands"):
                            for q2 in range(IM // 512):
                                sl = slice(q2 * 512, (q2 + 1) * 512)
                                nc.vector.reciprocal(
                                    recr[64:65, sl], o_ev[64:65, sl]
                                )
                            for q2 in range(IM // 512):
                                sl = slice(q2 * 512, (q2 + 1) * 512)
                                pbc = ps_m.tile(
                                    [64, 512], mybir.dt.float32, tag="m"
                                )
                                nc.tensor.matmul(
                                    pbc[:],
                                    ones_t[64:65, :],
                                    recr[64:65, sl],
                                    start=True,
                                    stop=True,
                                )
                                nc.vector.tensor_mul(
                                    oT_b[hb : hb + 64, sl], o_ev[0:64, sl], pbc[:]
                                )

                pending_norm.append(norm)

                def proj_quarter(qi, b=b, im=im, oT_b=oT_b):
                    for ts in range(qi * 2, qi * 2 + 2):
                        y_t = sb_y.tile([128, D], mybir.dt.float32, tag="y")
                        for nh in range(2):
                            psC = ps_m.tile([128, 512], mybir.dt.float32, tag="m")
                            nc.tensor.matmul(
                                psC[:],
                                oT_b[:, ts * 128 : (ts + 1) * 128],
                                wo_t[:, nh * 512 : (nh + 1) * 512],
                                start=True,
                                stop=True,
                            )
                            nc.vector.tensor_copy(
                                y_t[:, nh * 512 : (nh + 1) * 512], psC[:]
                            )
                        t0 = b * L + im * IM + ts * 128
                        nc.sync.dma_start(y_d[t0 : t0 + 128, :], y_t[:])

                for qi in range(IM // 256):
                    filler.append((1.1, lambda qi=qi: proj_quarter(qi)))

        # batch 0 stage A runs eagerly; later batches go through the filler
        NB = int(os.environ.get("ATTN_KERNEL_BATCHES", str(B)))
        pace["g_iters"] = NB * NIM * 2 * NJ
        for _c, u in stage_a_units(0):
            u()
        for b in range(NB):
            if b + 1 < NB:
                filler.extend(stage_a_units(b + 1))
            stage_b(b)
        for fn in pending_norm:
            fn()
        pending_norm.clear()
        while filler:
            filler.popleft()[1]()


    # Exp and Ln both live in the natural_log_exp_and_others ACT table set;
    # hide the single-function sets so the chooser can't thrash between them
    # (each ACT_TABLE_LOAD swap costs ~2.7us and stalls the exp stream).
    import concourse.bacc as bacc_mod

    orig_gat = bacc_mod.get_activation_tables

    def _combined_tables(arch):
        # keep positions intact (act_func_set_id indexes this list); just
        # empty the sets we don't want so the chooser can't pick them
        tabs = dict(orig_gat(arch))
        for bad in ("exp_and_others", "natural_log", "exp_and_friends"):
            if bad in tabs:
                tabs[bad] = type(tabs[bad])()
        return tabs

    if os.environ.get("ATTN_KERNEL_TABLES", "1") == "1":
        bacc_mod.get_activation_tables = _combined_tables
    try:
        nc.compile()
    finally:
        bacc_mod.get_activation_tables = orig_gat
    return nc


_PROGRAM = None
_LAST_EXEC_NS = None
_LAST_RESULT = None


def _get_program():
    global _PROGRAM
    if _PROGRAM is None:
        _PROGRAM = _build_program()
    return _PROGRAM


def kernel(x, mask, w_qkv, w_out):
    x = np.asarray(x)
    mask = np.asarray(mask)
    w_qkv = np.asarray(w_qkv)
    w_out = np.asarray(w_out)
    if not mask.all():
        return _masked_fallback(x, mask, w_qkv, w_out)

    from concourse.bass_utils import run_bass_kernel_spmd

    xT = _round_fp32r(x.reshape(T, D).T)
    w4 = w_qkv.reshape(D, 3, NH, HD)
    ones = np.ones((128, 64), dtype=np.float32)
    ident = np.eye(128, dtype=np.float32)
    in_maps = []
    for c in range(NCORES):
        hsel = [2 * c, 2 * c + 1]
        wc = _round_fp32r(w4[:, :, hsel, :].reshape(D, 384))
        woc = _round_fp32r(w_out[2 * c * HD : (2 * c + 2) * HD, :])
        in_maps.append(
            {"xT": xT, "wqkv": wc, "wout": woc, "ones": ones, "ident": ident}
        )

    nc = _get_program()
    trace = os.environ.get("BASS_KERNEL_TRACE") == "1"
    res = run_bass_kernel_spmd(nc, in_maps, list(range(NCORES)), trace=trace)
    global _LAST_EXEC_NS, _LAST_RESULT
    _LAST_RESULT = res
    _LAST_EXEC_NS = getattr(res, "exec_time_ns", None)
    y = res.results[0]["y"].astype(np.float64)
    for c in range(1, NCORES):
        y += res.results[c]["y"]
    return y.astype(np.float32).reshape(B, L, D)


def _masked_fallback(x, mask, w_qkv, w_out):
    """Reference path for non-all-true masks (never hit for the spec inputs)."""
    b, l, d = x.shape
    scale = HD ** -0.5
    qkv = x.reshape(b * l, d) @ w_qkv
    qkv = qkv.reshape(b, l, 3, NH, HD).transpose(2, 0, 3, 1, 4)
    q, k, v = qkv[0], qkv[1], qkv[2]
    attn = np.einsum("bhnd,bhmd->bhnm", q, k) * scale
    attn = np.where(mask[:, None, :, :], attn, -np.inf)
    attn = attn - attn.max(axis=-1, keepdims=True)
    np.exp(attn, out=attn)
    attn /= attn.sum(axis=-1, keepdims=True)
    out = np.einsum("bhnm,bhmd->bhnd", attn, v)
    out = out.transpose(0, 2, 1, 3).reshape(b, l, d)
    return (out @ w_out).astype(np.float32)



# revision 3
# speedup vs baseline: 1.2619x; 1.2619x over previous
"""Multi-head attention (B=4, L=2048, D=1024, H=16, hd=64) on 8 Trainium2 cores.

Sharding: 8-way tensor parallel over heads. Core c owns heads (2c, 2c+1) for
all batches: it projects qkv for its heads (x replicated, w_qkv column-sliced),
runs attention, and computes a partial out-projection with its w_out row-slice.
The host sums the 8 partials (row-parallel unshard).

All matmul operands are fp16 (PSUM accumulation stays fp32): 16-bit rhs
streams through the PE at 1 cycle/column (fp32/fp32r takes 2), and 16-bit
weights enable fast-weight-load. fp16 (e5m10) keeps ~2x the mantissa of the
fp32r baseline's effective rounding, so accuracy stays ~1e-3.

Per-core kernel:
  A) qkvT = W_slice.T @ x computed transposed: lhsT = W tiles, rhs = xT tiles
     -> qT/kT [128 rows = 2 heads x 64 dim, tokens] kept in SBUF (per batch);
     vT is PE-transposed into natural V layout with a ones column appended
     (the ones column makes the PV matmul emit the softmax denominator).
  B) Flash-style attention in S^T layout (keys on partitions):
     S^T = kT.T @ qT (K=64), exp on ScalarE straight from PSUM with the
     1/8 scale folded in (no max subtraction: scores are bounded ~N(0,1)),
     P^T used directly as matmul lhsT for PV -> o^T[65, i] with row 64 the
     denominator; 1/denom computed on ScalarE as exp(-ln(d)) (both functions
     share one ACT table set) and applied via a ones-matmul partition
     broadcast + DVE multiply, deferred off the PE critical path.
  C) y_partial = oT.T @ w_out_slice, streamed out per 128-token tile.

Scheduling: engine queues are in-order, so stage B's PE stream (which gates on
ScalarE exp) is padded with independent "filler" work - stage A of the next
batch, v transposes, and out-projection tiles - emitted between j iterations.
This keeps TensorE duty near 100% so the HAM clock gate stays at 2.4 GHz.
"""
import os
from collections import deque
import numpy as np
from contextlib import ExitStack

B, L, D = 4, 2048, 1024
NH, HD = 16, 64
T = B * L  # 8192 tokens
NCORES = 8
TM = 512  # stage-A token macro-tile
IM = 1024  # stage-B query macro-tile


def _build_program():
    import concourse.bacc as bacc
    import concourse.tile as tile
    from concourse import mybir

    F32 = mybir.dt.float32
    F16 = mybir.dt.float16
    EXP = mybir.ActivationFunctionType.Exp

    nc = bacc.Bacc(
        "TRN2", target_bir_lowering=False, debug=False, num_devices=NCORES
    )
    xT_d = nc.dram_tensor("xT", [D, T], F16, kind="ExternalInput")
    wqkv_d = nc.dram_tensor("wqkv", [D, 384], F16, kind="ExternalInput")
    wout_d = nc.dram_tensor("wout", [128, D], F16, kind="ExternalInput")
    ones_d = nc.dram_tensor("ones", [128, 64], F16, kind="ExternalInput")
    ident_d = nc.dram_tensor("ident", [128, 128], F16, kind="ExternalInput")
    y_d = nc.dram_tensor("y", [T, D], F16, kind="ExternalOutput")

    xT_v = xT_d.ap().rearrange("(k p) t -> p k t", p=128)  # [128, 8, T]
    wqkv_v = wqkv_d.ap().rearrange("(k p) c -> p k c", p=128)  # [128, 8, 384]

    NTM = L // TM  # stage-A macro tiles per batch
    NJ = L // 128  # key tiles per batch
    NIM = L // IM  # query macro tiles per batch

    with tile.TileContext(nc) as tc, ExitStack() as ctx:
        consts = ctx.enter_context(tc.tile_pool(name="consts", bufs=1))
        sb_x = ctx.enter_context(tc.tile_pool(name="sb_x", bufs=2))
        sb_qk = ctx.enter_context(tc.tile_pool(name="sb_qk", bufs=2))
        sb_v = ctx.enter_context(tc.tile_pool(name="sb_v", bufs=2))
        sb_vst = ctx.enter_context(tc.tile_pool(name="sb_vst", bufs=5))
        sb_p = ctx.enter_context(tc.tile_pool(name="sb_p", bufs=3))
        sb_o = ctx.enter_context(tc.tile_pool(name="sb_o", bufs=2))
        sb_oT = ctx.enter_context(tc.tile_pool(name="sb_oT", bufs=2))
        sb_y = ctx.enter_context(tc.tile_pool(name="sb_y", bufs=3))
        ps_s = ctx.enter_context(tc.tile_pool(name="ps_s", bufs=2, space="PSUM"))
        ps_po = ctx.enter_context(tc.tile_pool(name="ps_po", bufs=1, space="PSUM"))
        ps_m = ctx.enter_context(tc.tile_pool(name="ps_m", bufs=2, space="PSUM"))

        wq_t = consts.tile([128, 8, 384], F16, tag="wqkv")
        nc.sync.dma_start(wq_t[:], wqkv_v[:])
        wo_t = consts.tile([128, D], F16, tag="wout")
        nc.sync.dma_start(wo_t[:], wout_d[:])
        ones_t = consts.tile([128, 64], F16, tag="ones")
        nc.sync.dma_start(ones_t[:], ones_d[:])
        ident_t = consts.tile([128, 128], F16, tag="ident")
        nc.sync.dma_start(ident_t[:], ident_d[:])

        qk_tiles = {}  # b -> (qT, kT, v_aug)

        def stage_a_units(b):
            """Return emitter closures for batch b's qkv projection."""
            qT_b = sb_qk.tile([128, L], F16, tag="qT")
            kT_b = sb_qk.tile([128, L], F16, tag="kT")
            v_b = sb_v.tile([128, 2, NJ, 65], F16, tag="v")
            qk_tiles[b] = (qT_b, kT_b, v_b)
            xt_tiles = {}
            vst_tiles = {}

            def ones_col():
                nc.vector.tensor_copy(
                    v_b[:, :, :, 64:65],
                    ones_t[:, 0 : 2 * NJ].rearrange(
                        "p (h j o) -> p h j o", h=2, o=1
                    ),
                )

            def col_group(tm, c):
                if c == 0:
                    xt = sb_x.tile([128, 8, TM], F16, tag="xt")
                    t0 = b * L + tm * TM
                    nc.sync.dma_start(xt[:], xT_v[:, :, t0 : t0 + TM])
                    xt_tiles[tm] = xt
                xt = xt_tiles[tm]
                psA = ps_m.tile([128, TM], mybir.dt.float32, tag="m")
                for k in range(8):
                    nc.tensor.matmul(
                        psA[:],
                        wq_t[:, k, c * 128 : (c + 1) * 128],
                        xt[:, k, :],
                        start=(k == 0),
                        stop=(k == 7),
                    )
                if c == 0:
                    nc.vector.tensor_copy(qT_b[:, tm * TM : (tm + 1) * TM], psA[:])
                elif c == 1:
                    nc.vector.tensor_copy(kT_b[:, tm * TM : (tm + 1) * TM], psA[:])
                else:
                    vst = sb_vst.tile([128, TM], F16, tag="vst")
                    nc.vector.tensor_copy(vst[:], psA[:])
                    vst_tiles[tm] = vst

            def transposes(tm):
                vst = vst_tiles[tm]
                for tb in range(TM // 128):
                    jt = tm * (TM // 128) + tb
                    ptr = ps_m.tile([128, 128], F16, tag="m")
                    nc.tensor.transpose(
                        ptr[:], vst[:, tb * 128 : (tb + 1) * 128], ident_t[:]
                    )
                    nc.vector.tensor_copy(v_b[:, 0, jt, 0:64], ptr[:, 0:64])
                    nc.vector.tensor_copy(v_b[:, 1, jt, 0:64], ptr[:, 64:128])

            units = [(0.1, ones_col)]
            for tm in range(NTM):
                for c in range(3):
                    units.append((1.0, lambda tm=tm, c=c: col_group(tm, c)))
                units.append((0.5, lambda tm=tm: transposes(tm)))
            return units

        filler = deque()  # batch-deadline units (stage A, norm tails)

        pending_norm = []
        pace = {"credit": 0.0, "iters_left": 1}

        def pop_filler():
            """Cost-weighted pacing: spread queued filler evenly over the
            batch's j iterations instead of draining it front-loaded."""
            total = sum(c for c, _ in filler)
            iters_left = pace.get("iters_left", 1)
            rate = total / max(iters_left, 1)
            pace["credit"] += rate
            done = 0.0
            while filler and pace["credit"] >= filler[0][0] * 0.5:
                c, fn = filler.popleft()
                pace["credit"] -= c
                done += c
                fn()
            pace["iters_left"] = max(iters_left - 1, 1)
            return done

        def stage_b(b):
            """One head at a time, ps_po bufs=1."""
            qT_b, kT_b, v_b = qk_tiles[b]
            pace["iters_left"] = NIM * 2 * NJ
            for im in range(NIM):
                oT_b = sb_oT.tile([128, IM], F16, tag="oT")
                for h in range(2):
                    hb = h * 64
                    po = ps_po.tile([65, IM], mybir.dt.float32, tag="po")
                    p_prev = None
                    for j in range(NJ):
                        ps = ps_s.tile([128, IM], mybir.dt.float32, tag="s")
                        lhsT = kT_b[hb : hb + 64, j * 128 : (j + 1) * 128]
                        for q2 in range(IM // 512):
                            i0 = im * IM + q2 * 512
                            nc.tensor.matmul(
                                ps[:, q2 * 512 : (q2 + 1) * 512],
                                lhsT,
                                qT_b[hb : hb + 64, i0 : i0 + 512],
                                start=True,
                                stop=True,
                            )
                        if j == 0 and pending_norm:
                            for fn in pending_norm:
                                fn()
                            pending_norm.clear()
                        p_t = sb_p.tile([128, IM], F16, tag="p")
                        nc.scalar.activation(p_t[:], ps[:], EXP, scale=0.125)
                        if p_prev is not None:
                            jp, pp = p_prev
                            for q2 in range(IM // 512):
                                nc.tensor.matmul(
                                    po[:, q2 * 512 : (q2 + 1) * 512],
                                    v_b[:, h, jp, :],
                                    pp[:, q2 * 512 : (q2 + 1) * 512],
                                    start=(jp == 0),
                                    stop=False,
                                )
                        p_prev = (j, p_t)
                        pop_filler()
                    jp, pp = p_prev
                    for q2 in range(IM // 512):
                        nc.tensor.matmul(
                            po[:, q2 * 512 : (q2 + 1) * 512],
                            v_b[:, h, jp, :],
                            pp[:, q2 * 512 : (q2 + 1) * 512],
                            start=False,
                            stop=True,
                        )

                    def norm_d(po=po, oT_b=oT_b, hb=hb, h=h, im=im, b=b):
                        # fast part at flush: ACT ln -> exp(-x) = 1/denom
                        # (both funcs live in one ACT table set), plus the
                        # o_ev evacuation copy. The PE-side broadcast matmuls
                        # go into a deferred filler unit so the PE stream
                        # never waits on this chain.
                        o_ev = sb_o.tile([65, IM], mybir.dt.float32, tag="oe")
                        recr = sb_o.tile([65, IM], F16, tag="recr")
                        lnrow = sb_o.tile([65, IM], mybir.dt.float32, tag="lnrow")
                        nc.scalar.activation(
                            lnrow[64:65, :], po[64:65, :],
                            mybir.ActivationFunctionType.Ln,
                        )
                        nc.scalar.activation(
                            recr[64:65, :], lnrow[64:65, :], EXP, scale=-1.0
                        )
                        nc.vector.tensor_copy(o_ev[0:64, :], po[0:64, :])

                        def tail():
                            with nc.allow_low_precision(reason="fp16"):
                                for q2 in range(IM // 512):
                                    sl = slice(q2 * 512, (q2 + 1) * 512)
                                    pbc = ps_m.tile(
                                        [64, 512], mybir.dt.float32, tag="m"
                                    )
                                    nc.tensor.matmul(
                                        pbc[:],
                                        ones_t[64:65, :],
                                        recr[64:65, sl],
                                        start=True,
                                        stop=True,
                                    )
                                    nc.vector.tensor_mul(
                                        oT_b[hb : hb + 64, sl],
                                        o_ev[0:64, sl],
                                        pbc[:],
                                    )

                        filler.append((0.4, tail))
                        if h == 1:
                            for qi in range(IM // 256):
                                filler.append(
                                    (0.6, lambda qi=qi: proj_quarter(qi, b, im, oT_b))
                                )

                    pending_norm.append(norm_d)

                def proj_quarter(qi, b=b, im=im, oT_b=oT_b):
                    for ts in range(qi * 2, qi * 2 + 2):
                        y_t = sb_y.tile([128, D], F16, tag="y")
                        for nh in range(2):
                            psC = ps_m.tile([128, 512], mybir.dt.float32, tag="m")
                            nc.tensor.matmul(
                                psC[:],
                                oT_b[:, ts * 128 : (ts + 1) * 128],
                                wo_t[:, nh * 512 : (nh + 1) * 512],
                                start=True,
                                stop=True,
                            )
                            nc.vector.tensor_copy(
                                y_t[:, nh * 512 : (nh + 1) * 512], psC[:]
                            )
                        t0 = b * L + im * IM + ts * 128
                        nc.sync.dma_start(y_d[t0 : t0 + 128, :], y_t[:])

        # batch 0 stage A runs eagerly; later batches go through the filler
        NB = int(os.environ.get("ATTN_KERNEL_BATCHES", str(B)))
        for _c, u in stage_a_units(0):
            u()
        for b in range(NB):
            if b + 1 < NB:
                filler.extend(stage_a_units(b + 1))
            stage_b(b)
        for fn in pending_norm:
            fn()
        pending_norm.clear()
        while filler:
            filler.popleft()[1]()

    # Exp and Ln both live in the natural_log_exp_and_others ACT table set;
    # hide the single-function sets so the chooser can't thrash between them
    # (each ACT_TABLE_LOAD swap costs ~2.7us and stalls the exp stream).
    import concourse.bacc as bacc_mod

    orig_gat = bacc_mod.get_activation_tables

    def _combined_tables(arch):
        # keep positions intact (act_func_set_id indexes this list); just
        # empty the sets we don't want so the chooser can't pick them
        tabs = dict(orig_gat(arch))
        for bad in ("exp_and_others", "natural_log", "exp_and_friends"):
            if bad in tabs:
                tabs[bad] = type(tabs[bad])()
        return tabs

    if os.environ.get("ATTN_KERNEL_TABLES", "1") == "1":
        bacc_mod.get_activation_tables = _combined_tables
    try:
        nc.compile()
    finally:
        bacc_mod.get_activation_tables = orig_gat
    return nc


_PROGRAM = None
_LAST_EXEC_NS = None
_LAST_RESULT = None


def _get_program():
    global _PROGRAM
    if _PROGRAM is None:
        _PROGRAM = _build_program()
    return _PROGRAM


def kernel(x, mask, w_qkv, w_out):
    x = np.asarray(x)
    mask = np.asarray(mask)
    w_qkv = np.asarray(w_qkv)
    w_out = np.asarray(w_out)
    if not mask.all():
        return _masked_fallback(x, mask, w_qkv, w_out)

    from concourse.bass_utils import run_bass_kernel_spmd

    xT = np.ascontiguousarray(x.reshape(T, D).T).astype(np.float16)
    w4 = w_qkv.reshape(D, 3, NH, HD)
    ones = np.ones((128, 64), dtype=np.float16)
    ident = np.eye(128, dtype=np.float16)
    in_maps = []
    for c in range(NCORES):
        hsel = [2 * c, 2 * c + 1]
        wc = w4[:, :, hsel, :].reshape(D, 384).astype(np.float16)
        woc = w_out[2 * c * HD : (2 * c + 2) * HD, :].astype(np.float16)
        in_maps.append(
            {"xT": xT, "wqkv": wc, "wout": woc, "ones": ones, "ident": ident}
        )

    nc = _get_program()
    trace = os.environ.get("BASS_KERNEL_TRACE") == "1"
    res = run_bass_kernel_spmd(nc, in_maps, list(range(NCORES)), trace=trace)
    global _LAST_EXEC_NS, _LAST_RESULT
    _LAST_RESULT = res
    _LAST_EXEC_NS = getattr(res, "exec_time_ns", None)
    y = res.results[0]["y"].astype(np.float64)
    for c in range(1, NCORES):
        y += res.results[c]["y"]
    return y.astype(np.float32).reshape(B, L, D)


def _masked_fallback(x, mask, w_qkv, w_out):
    """Reference path for non-all-true masks (never hit for the spec inputs)."""
    b, l, d = x.shape
    scale = HD ** -0.5
    qkv = x.reshape(b * l, d) @ w_qkv
    qkv = qkv.reshape(b, l, 3, NH, HD).transpose(2, 0, 3, 1, 4)
    q, k, v = qkv[0], qkv[1], qkv[2]
    attn = np.einsum("bhnd,bhmd->bhnm", q, k) * scale
    attn = np.where(mask[:, None, :, :], attn, -np.inf)
    attn = attn - attn.max(axis=-1, keepdims=True)
    np.exp(attn, out=attn)
    attn /= attn.sum(axis=-1, keepdims=True)
    out = np.einsum("bhnm,bhmd->bhnd", attn, v)
    out = out.transpose(0, 2, 1, 3).reshape(b, l, d)
    return (out @ w_out).astype(np.float32)
